# revision 45
# baseline (speedup 1.0000x reference)
"""Trainium2 kernel for the scatter_memory problem.

Strategy (8 NeuronCores, expert-parallel over classes):
  * Classes are snake-assigned to 8 cores by descending instance count so the
    per-slot padded sizes are nearly uniform across cores (one SPMD program).
  * Each core receives a packed, transposed "candidates" matrix
    cand_T [D, W] (bf16): for each of its class slots, 10 memory-bank columns
    followed by that class's instances (original scan order), zero padded.
  * On device (per core):
      - dist[81, W]  = mem_mean @ cand: dot of every class mean with every
        candidate column -> drives the argmin over classes.
      - per class slot: gram[CAND, K] = cand_cls @ X_cls^T: dot tables that
        drive the order-dependent bank update.
    All matmuls bf16 inputs with fp32 PSUM accumulation.
  * On CPU: the tiny order-dependent bookkeeping. All comparisons use the
    device dot products; any decision whose margin is within TAU of the
    boundary is recomputed at higher precision (f32 sgemm, then f64), making
    every argmin/argmax decision exactly the true (f64) decision while the
    device does ~all the FLOPs and data movement. Decision margins for this
    problem's data are ~0.1-100 in squared-distance units; bf16 device dots
    are accurate to ~0.4, f32 to ~2e-4, so the two-level guard bands leave
    >5x safety at each level.

new_mem rows are bit-copies of input rows (instances / memory), so outputs
match the reference bit-exactly once every decision matches.
"""

import sys
import types
import numpy as np
import ml_dtypes

import concourse.mybir as mybir
import concourse.tile as tile
from concourse import bacc
from concourse.bass_utils import run_bass_kernel_spmd

N_CLASSES = 81
BANK = 10
D = 2048
N_CORES = 8
P = 128

TAU_DIST = 1.5    # bf16 distance d2 margin guard (measured dev err <= ~0.4)
TAU_DIST2 = 0.02  # f32 sgemm second-level guard (err ~2e-4)
TAU_GRAM = 6.0    # bf16 gram d2 margin guard

_PROGRAM_CACHE = {}


def _install_ntff_hook():
    """The agent image's antenv lacks axon_hooks; synthesize it so
    run_bass_kernel_spmd(trace=True) can profile via the axon .so."""
    name = "antenv.axon_hooks"
    mod = sys.modules.get(name)
    if mod is None:
        mod = types.ModuleType(name)
        mod._hook = None
        mod.set_axon_ntff_profile_hook = lambda h: setattr(mod, "_hook", h)
        mod.get_axon_ntff_profile_hook = lambda: mod._hook
        sys.modules[name] = mod
        try:
            import antenv
            antenv.axon_hooks = mod
        except ImportError:
            pass
    if mod.get_axon_ntff_profile_hook() is None:
        try:
            from trn_agent_boot.trn_boot import _ntff_profile_via_ctypes
            mod.set_axon_ntff_profile_hook(
                _ntff_profile_via_ctypes("/opt/axon/libaxon_pjrt.so"))
        except Exception:
            pass


def _assign_classes(counts):
    """Snake-assign classes (desc. by count) to cores; per-slot padded sizes."""
    order = np.argsort(-counts, kind="stable")
    n_slots = (N_CLASSES + N_CORES - 1) // N_CORES
    slot_classes = -np.ones((n_slots, N_CORES), dtype=np.int64)
    for r, cls in enumerate(order):
        j, i = divmod(r, N_CORES)
        core = i if (j % 2 == 0) else N_CORES - 1 - i
        slot_classes[j, core] = cls
    k_pad = []
    for j in range(n_slots):
        mx = max(int(counts[c]) if c >= 0 else 0 for c in slot_classes[j])
        k_pad.append(max(8, (mx + 7) // 8 * 8))
    return slot_classes, k_pad


def _build_program(w_pad, k_pads, cands, offs):
    n_slots = len(k_pads)
    kch = D // P
    nc = bacc.Bacc("TRN2", target_bir_lowering=False, debug=False)
    cand_dram = nc.dram_tensor("cand_t", [D, w_pad], mybir.dt.bfloat16,
                               kind="ExternalInput")
    # mem_mean, pre-arranged host-side to [P, KCH*C] so the DMA is contiguous
    mm_dram = nc.dram_tensor("mm_t", [P, kch * N_CLASSES], mybir.dt.bfloat16,
                             kind="ExternalInput")
    dist_dram = nc.dram_tensor("dist", [N_CLASSES, w_pad], mybir.dt.float32,
                               kind="ExternalOutput")
    # gram tables packed tight into one [P, sum(k_pads per rblock)] tensor:
    # rblock (j, r0) owns columns [gcol[j,r0], gcol[j,r0]+k_pads[j])
    rblocks = [(j, r0) for j in range(n_slots) for r0 in range(0, cands[j], P)]
    gcols = {}
    gw = 0
    for j, r0 in rblocks:
        gcols[(j, r0)] = gw
        gw += k_pads[j] - max(0, r0 - BANK)
    gram_dram = nc.dram_tensor("gram", [P, gw], mybir.dt.float32,
                               kind="ExternalOutput")

    n_db = (w_pad + 511) // 512  # distance column blocks
    db_sizes = [min(512, w_pad - b * 512) for b in range(n_db)]

    kh = kch // 2                  # gram/dist contraction halves
    n_sub = 2                      # column-split per chunk DMA (2-wave arrival)
    sub = (w_pad // n_sub + P - 1) // P * P

    class _SlimExitTC(tile.TileContext):
        # Keep the stock exit protocol (drain + barrier + sem clear +
        # barrier) but use sequencer-only barriers: skips the per-engine
        # DRAIN round-trips while preserving the all-engine rendezvous.
        def _drain_and_barrier(self, tick_clock, wait_clock):
            nc_ = self.nc
            drain_inst = nc_.sync.drain()
            wait_clock.add_sem_waits(
                drain_inst.ins,
                tile.ScopedClock({None: tick_clock.global_clock}))
            nc_.all_engine_barrier(sem_only=True)
            popped = nc_._tile_sem_poison_stack.pop()
            assert popped is self._sem_poison
            nc_.clear_and_free_semaphores(list(self.sems.allocated().values()))
            nc_.all_engine_barrier(sem_only=True)

    with _SlimExitTC(nc) as tc:
        with (
            tc.tile_pool(name="bfpool", bufs=kch) as bfpool,
            tc.tile_pool(name="mmpool", bufs=1) as mmpool,
            tc.tile_pool(name="opool", bufs=4) as opool,
            tc.tile_pool(name="daccpool", bufs=n_db) as daccpool,
            tc.tile_pool(name="goutpool", bufs=1) as goutpool,
            tc.tile_pool(name="gpartpool", bufs=len(rblocks)) as gpartpool,
            tc.tile_pool(name="dps", bufs=n_db, space="PSUM") as dps,
            tc.tile_pool(name="gps", bufs=3, space="PSUM") as gps,
        ):
            mm_sb = mmpool.tile([P, kch, N_CLASSES], mybir.dt.bfloat16)
            nc.sync.dma_start(mm_sb[:], mm_dram.rearrange("p (k c) -> p k c",
                                                          k=kch))

            # PE warmup: dummy matmuls on the (small, early) mm tile so the
            # HAM clock gate opens before the real work arrives and stays
            # open through the chunk-arrival gaps.
            warm_ps = gps.tile([P, 512], mybir.dt.float32, tag="gps",
                               name="warm_ps")
            mm_flat = mm_sb.rearrange("p k c -> p (k c)")

            def warm(n):
                for _ in range(n):
                    nc.tensor.matmul(warm_ps[:, :512], mm_flat[:, :128],
                                     mm_flat[:, :512], start=True, stop=True)

            warm(10)

            cand_bf = [bfpool.tile([P, w_pad], mybir.dt.bfloat16, tag="candbf",
                                   name=f"cand{k}") for k in range(kch)]

            def chunk_dma(k):
                for s in range(n_sub):
                    c0 = s * sub
                    cw = min(sub, w_pad - c0)
                    if cw > 0:
                        nc.sync.dma_start(
                            cand_bf[k][:, c0:c0 + cw],
                            cand_dram[k * P:(k + 1) * P, c0:c0 + cw])

            def gram_mms(j, r0, klo, khi, suffix, pool=None, tag="gps"):
                # Triangular skip: candidate row r0+i (instance r0+i-BANK) is
                # only read at steps t > r0+i-BANK, so rowblocks with r0 > 0
                # only need columns >= r0-BANK.
                o = offs[j]
                kp = k_pads[j]
                rows = min(P, cands[j] - r0)
                c_lo = max(0, r0 - BANK)
                gp = (pool or gps).tile([P, 512], mybir.dt.float32, tag=tag,
                                        name=f"gps{suffix}{j}_{r0}")[:rows,
                                                                     :kp - c_lo]
                for k in range(klo, khi):
                    nc.tensor.matmul(
                        gp,
                        cand_bf[k][:, o + r0:o + r0 + rows],
                        cand_bf[k][:, o + BANK + c_lo:o + BANK + kp],
                        start=(k == klo), stop=(k == khi - 1),
                    )
                return gp, rows, kp - c_lo

            # ---- chunks 0..kh-1 stream in; dist half-0 paced by arrival ----
            dist_ps0 = [dps.tile([P, 512], mybir.dt.float32, tag="dps",
                                 name=f"dps0_{b}")[:N_CLASSES, :db_sizes[b]]
                        for b in range(n_db)]
            for k in range(kh):
                chunk_dma(k)
                for b in range(n_db):
                    nc.tensor.matmul(
                        dist_ps0[b], mm_sb[:, k, :],
                        cand_bf[k][:, b * 512:b * 512 + db_sizes[b]],
                        start=(k == 0), stop=(k == kh - 1))
            # issue the second half's DMAs right away so queues stay busy
            for k in range(kh, kch):
                chunk_dma(k)
            # drain dist half-0 psum to SBUF accumulators
            dacc = []
            for b in range(n_db):
                da = daccpool.tile([N_CLASSES, 512], mybir.dt.float32,
                                   tag="dacc", name=f"dacc{b}")
                nc.scalar.copy(da[:, :db_sizes[b]], dist_ps0[b])
                dacc.append(da)

            # ---- gram half-A (chunks 0..kh-1) overlaps chunk kh..15 DMAs ----
            partials = {}
            ai = 0
            for j, r0 in rblocks:
                ai += 1
                pool, tag = (gps, "gps") if ai % 2 else (dps, "dps")
                gp, rows, kp = gram_mms(j, r0, 0, kh, "A", pool, tag)
                ga = gpartpool.tile([P, 512], mybir.dt.float32, tag="gpart",
                                    name=f"gpart{j}_{r0}")
                nc.scalar.copy(ga[:rows, :kp], gp)
                partials[(j, r0)] = ga

            # ---- dist half-1 (all chunks present by now) ----
            dist_ps1 = [dps.tile([P, 512], mybir.dt.float32, tag="dps",
                                 name=f"dps1_{b}")[:N_CLASSES, :db_sizes[b]]
                        for b in range(n_db)]
            for k in range(kh, kch):
                for b in range(n_db):
                    nc.tensor.matmul(
                        dist_ps1[b], mm_sb[:, k, :],
                        cand_bf[k][:, b * 512:b * 512 + db_sizes[b]],
                        start=(k == kh), stop=(k == kch - 1))
            for b in range(n_db):
                dsb = opool.tile([N_CLASSES, 512], mybir.dt.float32, tag="dout")
                nc.vector.tensor_add(dsb[:, :db_sizes[b]], dist_ps1[b],
                                     dacc[b][:, :db_sizes[b]])
                nc.sync.dma_start(dist_dram[:, b * 512:b * 512 + db_sizes[b]],
                                  dsb[:, :db_sizes[b]])

            # ---- gram half-B + combine into the packed output tile ----
            gout = goutpool.tile([P, gw], mybir.dt.float32)
            nc.any.memzero(gout[:])
            n_og = 11
            per_g = (len(rblocks) + n_og - 1) // n_og
            bi = 0
            for gi in range(n_og):
                grp = rblocks[gi * per_g:(gi + 1) * per_g]
                if not grp:
                    continue
                for j, r0 in grp:
                    # alternate PSUM pools: dist's dps banks are free by now,
                    # giving 8 in-flight accumulators instead of 3
                    bi += 1
                    pool, tag = (gps, "gps") if bi % 2 else (dps, "dps")
                    gp, rows, kp = gram_mms(j, r0, kh, kch, "B", pool, tag)
                    ga = partials[(j, r0)]
                    gc = gcols[(j, r0)]
                    nc.vector.tensor_add(gout[:rows, gc:gc + kp], gp,
                                         ga[:rows, :kp])
                c_lo = gcols[grp[0]]
                j, r0 = grp[-1]
                c_hi = gcols[(j, r0)] + k_pads[j] - max(0, r0 - BANK)
                nc.sync.dma_start(gram_dram[:, c_lo:c_hi], gout[:, c_lo:c_hi])
    nc.compile()
    return nc


def _get_program(key_args):
    key = repr(key_args)
    if key not in _PROGRAM_CACHE:
        _PROGRAM_CACHE[key] = _build_program(*key_args)
    return _PROGRAM_CACHE[key]


def _f64_row_norms(x, chunk=4096):
    out = np.empty(x.shape[0], dtype=np.float64)
    for i in range(0, x.shape[0], chunk):
        xi = x[i:i + chunk].astype(np.float64)
        out[i:i + chunk] = np.einsum("ij,ij->i", xi, xi)
    return out


def kernel(instances, instance_labels, memory, memory_pos, _trace=False):
    instances = np.ascontiguousarray(instances, dtype=np.float32)
    labels = np.asarray(instance_labels).astype(np.int64)
    memory = np.ascontiguousarray(memory, dtype=np.float32)
    memory_pos = np.asarray(memory_pos)
    n_ins = instances.shape[0]

    counts = np.bincount(labels, minlength=N_CLASSES)
    slot_classes, k_pads = _assign_classes(counts)
    n_slots = len(k_pads)
    cands = [BANK + kp for kp in k_pads]
    offs = np.concatenate([[0], np.cumsum(cands)]).astype(int)
    w = int(offs[-1])
    w_pad = (w + P - 1) // P * P

    # Per-class instance index lists (original scan order).
    sort_idx = np.argsort(labels, kind="stable")
    cls_starts = np.concatenate([[0], np.cumsum(counts)]).astype(int)
    idx_by_class = [sort_idx[cls_starts[c]:cls_starts[c + 1]] for c in range(N_CLASSES)]

    ins_bf = instances.astype(ml_dtypes.bfloat16)
    mem_bf = memory.astype(ml_dtypes.bfloat16)
    mem_mean32 = memory.mean(axis=1)  # [C, D] f32
    # [D, C] -> chunked [P, KCH*C] so the device DMA is one contiguous copy
    mm_t = np.ascontiguousarray(
        mem_mean32.T.astype(ml_dtypes.bfloat16)
        .reshape(D // P, P, N_CLASSES).transpose(1, 0, 2)
        .reshape(P, (D // P) * N_CLASSES))

    # Build per-core packed transposed candidates (bf16).
    in_maps = []
    core_cls = [[] for _ in range(N_CORES)]  # (slot j, class c)
    for core in range(N_CORES):
        cand_rows = np.zeros((w_pad, D), dtype=ml_dtypes.bfloat16)
        for j in range(n_slots):
            c = slot_classes[j][core]
            if c < 0:
                continue
            core_cls[core].append((j, int(c)))
            o = offs[j]
            cand_rows[o:o + BANK] = mem_bf[c]
            kc = int(counts[c])
            cand_rows[o + BANK:o + BANK + kc] = ins_bf[idx_by_class[c]]
        in_maps.append({
            "cand_t": np.ascontiguousarray(cand_rows.T),
            "mm_t": mm_t,
        })

    nc = _get_program((w_pad, tuple(k_pads), tuple(cands), tuple(offs[:-1])))
    if _trace:
        _install_ntff_hook()
    res = run_bass_kernel_spmd(nc, in_maps, list(range(N_CORES)), trace=_trace)
    results = res.results
    if _trace:
        kernel.last_exec_time_ns = res.exec_time_ns
        kernel.last_results = results
        kernel.last_meta = (core_cls, [o for o in offs], idx_by_class)

    # ---------------- CPU bookkeeping (f64-exact decisions) ----------------
    norm_x = _f64_row_norms(instances)                       # [N]
    mem64 = memory.astype(np.float64)
    mm64 = mem64.mean(axis=1)                                # [C, D]
    norm_mm = np.einsum("cd,cd->c", mm64, mm64)              # [C]
    norm_slots = np.einsum("cbd,cbd->cb", mem64, mem64)      # [C, BANK]

    # --- argmin over classes ---
    # d2part[n, c] = |mm_c|^2 - 2 * dot(x_n, mm_c)   (|x|^2 is constant per row)
    d2part = np.empty((n_ins, N_CLASSES), dtype=np.float64)
    for core in range(N_CORES):
        dist = results[core]["dist"]  # [C, w_pad] f32
        for j, c in core_cls[core]:
            idx = idx_by_class[c]
            if len(idx) == 0:
                continue
            o = offs[j] + BANK
            d2part[idx, :] = norm_mm[None, :] - 2.0 * dist[:, o:o + len(idx)].T
    cls_prob = np.argmin(d2part, axis=1).astype(np.int32)
    part = np.partition(d2part, 1, axis=1)
    need = np.nonzero(part[:, 1] - part[:, 0] < TAU_DIST)[0]
    if len(need):
        # level 2: f32 sgemm
        d2f32 = (norm_mm[None, :].astype(np.float32)
                 - 2.0 * (instances[need] @ mem_mean32.T))
        cls_prob[need] = np.argmin(d2f32, axis=1).astype(np.int32)
        p2 = np.partition(d2f32.astype(np.float64), 1, axis=1)
        need2 = np.nonzero(p2[:, 1] - p2[:, 0] < TAU_DIST2)[0]
        if len(need2):
            xr = instances[need[need2]].astype(np.float64)
            d2ex = norm_mm[None, :] - 2.0 * (xr @ mm64.T)
            cls_prob[need[need2]] = np.argmin(d2ex, axis=1).astype(np.int32)
        kernel.n_refine_dist2 = len(need2)
    else:
        kernel.n_refine_dist2 = 0
    kernel.n_refine_dist = len(need)

    acc = np.float32(np.mean((cls_prob.astype(np.int64) == labels).astype(np.float32)))

    # --- sequential bank update, vectorized across classes in lockstep ---
    k_max = int(counts.max()) if n_ins else 0
    cand_max = max(cands)
    kp_max = max(k_pads)
    g_stack = np.zeros((N_CLASSES, cand_max, kp_max), dtype=np.float32)
    gcols = {}
    gw = 0
    for j in range(n_slots):
        for r0 in range(0, cands[j], P):
            gcols[(j, r0)] = gw
            gw += k_pads[j] - max(0, r0 - BANK)
    for core in range(N_CORES):
        gram = results[core]["gram"]  # [P, gw]
        for j, c in core_cls[core]:
            kp = k_pads[j]
            for r0 in range(0, cands[j], P):
                rows = min(P, cands[j] - r0)
                gc = gcols[(j, r0)]
                c_lo = max(0, r0 - BANK)
                g_stack[c, r0:r0 + rows, c_lo:kp] = gram[:rows,
                                                         gc:gc + kp - c_lo]

    src = np.tile(np.arange(BANK, dtype=np.int64), (N_CLASSES, 1))  # cand row idx
    snorm = norm_slots.copy()                                       # [C, BANK]
    pos = memory_pos.astype(np.int64).copy()
    n_refine_chain = 0
    for t in range(k_max):
        active = counts > t
        app = active & (pos < BANK)
        full = active & ~app
        if app.any():
            ac = np.nonzero(app)[0]
            j_new = pos[ac]
            g_idx = np.array([idx_by_class[c][t] for c in ac])
            src[ac, j_new] = BANK + t
            snorm[ac, j_new] = norm_x[g_idx]
            pos[ac] += 1
        if full.any():
            fc = np.nonzero(full)[0]
            dots = g_stack[fc[:, None], src[fc], t].astype(np.float64)  # [F, BANK]
            d2a = snorm[fc] - 2.0 * dots
            prt = np.partition(d2a, BANK - 2, axis=1)
            margin = prt[:, -1] - prt[:, -2]
            j_new = np.argmax(d2a, axis=1)
            for fi in np.nonzero(margin < TAU_GRAM)[0]:
                c = fc[fi]
                n_refine_chain += 1
                vecs = np.empty((BANK, D), dtype=np.float64)
                for jj in range(BANK):
                    s = src[c, jj]
                    if s < BANK:
                        vecs[jj] = mem64[c, s]
                    else:
                        vecs[jj] = instances[idx_by_class[c][s - BANK]]
                x_t = instances[idx_by_class[c][t]].astype(np.float64)
                d2ex = ((vecs - x_t[None, :]) ** 2).sum(axis=1)
                j_new[fi] = int(np.argmax(d2ex))
            g_idx = np.array([idx_by_class[c][t] for c in fc])
            src[fc, j_new] = BANK + t
            snorm[fc, j_new] = norm_x[g_idx]
    kernel.n_refine_chain = n_refine_chain

    new_mem = np.empty_like(memory)
    for c in range(N_CLASSES):
        for jj in range(BANK):
            s = src[c, jj]
            if s < BANK:
                new_mem[c, jj] = memory[c, s]
            else:
                new_mem[c, jj] = instances[idx_by_class[c][s - BANK]]
    new_pos = np.minimum(memory_pos.astype(np.int64) + counts, BANK).astype(
        memory_pos.dtype)

    return cls_prob, acc, new_mem, new_pos


# revision 46
# speedup vs baseline: 1.1849x; 1.1849x over previous
"""Trainium2 kernel for the scatter_memory problem.

Strategy (8 NeuronCores, expert-parallel over classes):
  * Classes are snake-assigned to 8 cores by descending instance count so the
    per-slot padded sizes are nearly uniform across cores (one SPMD program).
  * Each core receives a packed, transposed "candidates" matrix
    cand_T [D, W] (bf16): for each of its class slots, 10 memory-bank columns
    followed by that class's instances (original scan order), zero padded.
  * On device (per core):
      - dist[81, W]  = mem_mean @ cand: dot of every class mean with every
        candidate column -> drives the argmin over classes.
      - per class slot: gram[CAND, K] = cand_cls @ X_cls^T: dot tables that
        drive the order-dependent bank update.
    All matmuls bf16 inputs with fp32 PSUM accumulation.
  * On CPU: the tiny order-dependent bookkeeping. All comparisons use the
    device dot products; any decision whose margin is within TAU of the
    boundary is recomputed at higher precision (f32 sgemm, then f64), making
    every argmin/argmax decision exactly the true (f64) decision while the
    device does ~all the FLOPs and data movement. Decision margins for this
    problem's data are ~0.1-100 in squared-distance units; bf16 device dots
    are accurate to ~0.4, f32 to ~2e-4, so the two-level guard bands leave
    >5x safety at each level.

new_mem rows are bit-copies of input rows (instances / memory), so outputs
match the reference bit-exactly once every decision matches.
"""

import sys
import types
import numpy as np
import ml_dtypes

import concourse.mybir as mybir
import concourse.tile as tile
from concourse import bacc
from concourse.bass_utils import run_bass_kernel_spmd

N_CLASSES = 81
BANK = 10
D = 2048
N_CORES = 8
P = 128

TAU_DIST = 1.5    # bf16 distance d2 margin guard (measured dev err <= ~0.4)
TAU_DIST2 = 0.02  # f32 sgemm second-level guard (err ~2e-4)
TAU_GRAM = 6.0    # bf16 gram d2 margin guard

_PROGRAM_CACHE = {}


def _install_ntff_hook():
    """The agent image's antenv lacks axon_hooks; synthesize it so
    run_bass_kernel_spmd(trace=True) can profile via the axon .so."""
    name = "antenv.axon_hooks"
    mod = sys.modules.get(name)
    if mod is None:
        mod = types.ModuleType(name)
        mod._hook = None
        mod.set_axon_ntff_profile_hook = lambda h: setattr(mod, "_hook", h)
        mod.get_axon_ntff_profile_hook = lambda: mod._hook
        sys.modules[name] = mod
        try:
            import antenv
            antenv.axon_hooks = mod
        except ImportError:
            pass
    if mod.get_axon_ntff_profile_hook() is None:
        try:
            from trn_agent_boot.trn_boot import _ntff_profile_via_ctypes
            mod.set_axon_ntff_profile_hook(
                _ntff_profile_via_ctypes("/opt/axon/libaxon_pjrt.so"))
        except Exception:
            pass


def _assign_classes(counts):
    """Snake-assign classes (desc. by count) to cores; per-slot padded sizes."""
    order = np.argsort(-counts, kind="stable")
    n_slots = (N_CLASSES + N_CORES - 1) // N_CORES
    slot_classes = -np.ones((n_slots, N_CORES), dtype=np.int64)
    for r, cls in enumerate(order):
        j, i = divmod(r, N_CORES)
        core = i if (j % 2 == 0) else N_CORES - 1 - i
        slot_classes[j, core] = cls
    k_pad = []
    for j in range(n_slots):
        mx = max(int(counts[c]) if c >= 0 else 0 for c in slot_classes[j])
        k_pad.append(max(8, (mx + 7) // 8 * 8))
    return slot_classes, k_pad


def _build_program(w_pad, k_pads, cands, offs):
    n_slots = len(k_pads)
    kch = D // P
    nc = bacc.Bacc("TRN2", target_bir_lowering=False, debug=False)
    cand_dram = nc.dram_tensor("cand_t", [D, w_pad], mybir.dt.bfloat16,
                               kind="ExternalInput")
    # mem_mean, pre-arranged host-side to [P, KCH*C] so the DMA is contiguous
    mm_dram = nc.dram_tensor("mm_t", [P, kch * N_CLASSES], mybir.dt.bfloat16,
                             kind="ExternalInput")
    dist_dram = nc.dram_tensor("dist", [N_CLASSES, w_pad], mybir.dt.float32,
                               kind="ExternalOutput")
    # gram tables packed tight into one [P, sum(k_pads per rblock)] tensor:
    # rblock (j, r0) owns columns [gcol[j,r0], gcol[j,r0]+k_pads[j])
    rblocks = [(j, r0) for j in range(n_slots) for r0 in range(0, cands[j], P)]
    gcols = {}
    gw = 0
    for j, r0 in rblocks:
        gcols[(j, r0)] = gw
        gw += k_pads[j] - max(0, r0 - BANK)
    gram_dram = nc.dram_tensor("gram", [P, gw], mybir.dt.float32,
                               kind="ExternalOutput")

    n_db = (w_pad + 511) // 512  # distance column blocks
    db_sizes = [min(512, w_pad - b * 512) for b in range(n_db)]

    kh = kch // 2                  # gram/dist contraction halves
    n_sub = 2                      # column-split per chunk DMA (2-wave arrival)
    sub = (w_pad // n_sub + P - 1) // P * P

    with tile.TileContext(nc) as tc:
        with (
            tc.tile_pool(name="bfpool", bufs=kch) as bfpool,
            tc.tile_pool(name="mmpool", bufs=1) as mmpool,
            tc.tile_pool(name="opool", bufs=4) as opool,
            tc.tile_pool(name="daccpool", bufs=n_db) as daccpool,
            tc.tile_pool(name="goutpool", bufs=1) as goutpool,
            tc.tile_pool(name="gpartpool", bufs=len(rblocks)) as gpartpool,
            tc.tile_pool(name="dps", bufs=n_db, space="PSUM") as dps,
            tc.tile_pool(name="gps", bufs=3, space="PSUM") as gps,
        ):
            mm_sb = mmpool.tile([P, kch, N_CLASSES], mybir.dt.bfloat16)
            nc.sync.dma_start(mm_sb[:], mm_dram.rearrange("p (k c) -> p k c",
                                                          k=kch))

            # PE warmup: dummy matmuls on the (small, early) mm tile so the
            # HAM clock gate opens before the real work arrives and stays
            # open through the chunk-arrival gaps.
            warm_ps = gps.tile([P, 512], mybir.dt.float32, tag="gps",
                               name="warm_ps")
            mm_flat = mm_sb.rearrange("p k c -> p (k c)")

            def warm(n):
                for _ in range(n):
                    nc.tensor.matmul(warm_ps[:, :512], mm_flat[:, :128],
                                     mm_flat[:, :512], start=True, stop=True)

            warm(10)

            cand_bf = [bfpool.tile([P, w_pad], mybir.dt.bfloat16, tag="candbf",
                                   name=f"cand{k}") for k in range(kch)]

            def chunk_dma(k):
                for s in range(n_sub):
                    c0 = s * sub
                    cw = min(sub, w_pad - c0)
                    if cw > 0:
                        nc.sync.dma_start(
                            cand_bf[k][:, c0:c0 + cw],
                            cand_dram[k * P:(k + 1) * P, c0:c0 + cw])

            def gram_mms(j, r0, klo, khi, suffix, pool=None, tag="gps"):
                # Triangular skip: candidate row r0+i (instance r0+i-BANK) is
                # only read at steps t > r0+i-BANK, so rowblocks with r0 > 0
                # only need columns >= r0-BANK.
                o = offs[j]
                kp = k_pads[j]
                rows = min(P, cands[j] - r0)
                c_lo = max(0, r0 - BANK)
                gp = (pool or gps).tile([P, 512], mybir.dt.float32, tag=tag,
                                        name=f"gps{suffix}{j}_{r0}")[:rows,
                                                                     :kp - c_lo]
                for k in range(klo, khi):
                    nc.tensor.matmul(
                        gp,
                        cand_bf[k][:, o + r0:o + r0 + rows],
                        cand_bf[k][:, o + BANK + c_lo:o + BANK + kp],
                        start=(k == klo), stop=(k == khi - 1),
                    )
                return gp, rows, kp - c_lo

            # ---- chunks 0..kh-1 stream in; dist half-0 paced by arrival ----
            dist_ps0 = [dps.tile([P, 512], mybir.dt.float32, tag="dps",
                                 name=f"dps0_{b}")[:N_CLASSES, :db_sizes[b]]
                        for b in range(n_db)]
            for k in range(kh):
                chunk_dma(k)
                for b in range(n_db):
                    nc.tensor.matmul(
                        dist_ps0[b], mm_sb[:, k, :],
                        cand_bf[k][:, b * 512:b * 512 + db_sizes[b]],
                        start=(k == 0), stop=(k == kh - 1))
            # issue the second half's DMAs right away so queues stay busy
            for k in range(kh, kch):
                chunk_dma(k)
            # drain dist half-0 psum to SBUF accumulators
            dacc = []
            for b in range(n_db):
                da = daccpool.tile([N_CLASSES, 512], mybir.dt.float32,
                                   tag="dacc", name=f"dacc{b}")
                nc.scalar.copy(da[:, :db_sizes[b]], dist_ps0[b])
                dacc.append(da)

            # ---- gram half-A (chunks 0..kh-1) overlaps chunk kh..15 DMAs ----
            partials = {}
            ai = 0
            for j, r0 in rblocks:
                ai += 1
                pool, tag = (gps, "gps") if ai % 2 else (dps, "dps")
                gp, rows, kp = gram_mms(j, r0, 0, kh, "A", pool, tag)
                ga = gpartpool.tile([P, 512], mybir.dt.float32, tag="gpart",
                                    name=f"gpart{j}_{r0}")
                nc.scalar.copy(ga[:rows, :kp], gp)
                partials[(j, r0)] = ga

            # ---- dist half-1 (all chunks present by now) ----
            dist_ps1 = [dps.tile([P, 512], mybir.dt.float32, tag="dps",
                                 name=f"dps1_{b}")[:N_CLASSES, :db_sizes[b]]
                        for b in range(n_db)]
            for k in range(kh, kch):
                for b in range(n_db):
                    nc.tensor.matmul(
                        dist_ps1[b], mm_sb[:, k, :],
                        cand_bf[k][:, b * 512:b * 512 + db_sizes[b]],
                        start=(k == kh), stop=(k == kch - 1))
            for b in range(n_db):
                dsb = opool.tile([N_CLASSES, 512], mybir.dt.float32, tag="dout")
                nc.vector.tensor_add(dsb[:, :db_sizes[b]], dist_ps1[b],
                                     dacc[b][:, :db_sizes[b]])
                nc.sync.dma_start(dist_dram[:, b * 512:b * 512 + db_sizes[b]],
                                  dsb[:, :db_sizes[b]])

            # ---- gram half-B + combine into the packed output tile ----
            gout = goutpool.tile([P, gw], mybir.dt.float32)
            nc.any.memzero(gout[:])
            n_og = 11
            per_g = (len(rblocks) + n_og - 1) // n_og
            bi = 0
            for gi in range(n_og):
                grp = rblocks[gi * per_g:(gi + 1) * per_g]
                if not grp:
                    continue
                for j, r0 in grp:
                    # alternate PSUM pools: dist's dps banks are free by now,
                    # giving 8 in-flight accumulators instead of 3
                    bi += 1
                    pool, tag = (gps, "gps") if bi % 2 else (dps, "dps")
                    gp, rows, kp = gram_mms(j, r0, kh, kch, "B", pool, tag)
                    ga = partials[(j, r0)]
                    gc = gcols[(j, r0)]
                    nc.vector.tensor_add(gout[:rows, gc:gc + kp], gp,
                                         ga[:rows, :kp])
                c_lo = gcols[grp[0]]
                j, r0 = grp[-1]
                c_hi = gcols[(j, r0)] + k_pads[j] - max(0, r0 - BANK)
                nc.sync.dma_start(gram_dram[:, c_lo:c_hi], gout[:, c_lo:c_hi])
    nc.compile()
    return nc


def _get_program(key_args):
    key = repr(key_args)
    if key not in _PROGRAM_CACHE:
        _PROGRAM_CACHE[key] = _build_program(*key_args)
    return _PROGRAM_CACHE[key]


def _f64_row_norms(x, chunk=4096):
    out = np.empty(x.shape[0], dtype=np.float64)
    for i in range(0, x.shape[0], chunk):
        xi = x[i:i + chunk].astype(np.float64)
        out[i:i + chunk] = np.einsum("ij,ij->i", xi, xi)
    return out


def kernel(instances, instance_labels, memory, memory_pos, _trace=False):
    instances = np.ascontiguousarray(instances, dtype=np.float32)
    labels = np.asarray(instance_labels).astype(np.int64)
    memory = np.ascontiguousarray(memory, dtype=np.float32)
    memory_pos = np.asarray(memory_pos)
    n_ins = instances.shape[0]

    counts = np.bincount(labels, minlength=N_CLASSES)
    slot_classes, k_pads = _assign_classes(counts)
    n_slots = len(k_pads)
    cands = [BANK + kp for kp in k_pads]
    offs = np.concatenate([[0], np.cumsum(cands)]).astype(int)
    w = int(offs[-1])
    w_pad = (w + P - 1) // P * P

    # Per-class instance index lists (original scan order).
    sort_idx = np.argsort(labels, kind="stable")
    cls_starts = np.concatenate([[0], np.cumsum(counts)]).astype(int)
    idx_by_class = [sort_idx[cls_starts[c]:cls_starts[c + 1]] for c in range(N_CLASSES)]

    ins_bf = instances.astype(ml_dtypes.bfloat16)
    mem_bf = memory.astype(ml_dtypes.bfloat16)
    mem_mean32 = memory.mean(axis=1)  # [C, D] f32
    # [D, C] -> chunked [P, KCH*C] so the device DMA is one contiguous copy
    mm_t = np.ascontiguousarray(
        mem_mean32.T.astype(ml_dtypes.bfloat16)
        .reshape(D // P, P, N_CLASSES).transpose(1, 0, 2)
        .reshape(P, (D // P) * N_CLASSES))

    # Build per-core packed transposed candidates (bf16).
    in_maps = []
    core_cls = [[] for _ in range(N_CORES)]  # (slot j, class c)
    for core in range(N_CORES):
        cand_rows = np.zeros((w_pad, D), dtype=ml_dtypes.bfloat16)
        for j in range(n_slots):
            c = slot_classes[j][core]
            if c < 0:
                continue
            core_cls[core].append((j, int(c)))
            o = offs[j]
            cand_rows[o:o + BANK] = mem_bf[c]
            kc = int(counts[c])
            cand_rows[o + BANK:o + BANK + kc] = ins_bf[idx_by_class[c]]
        in_maps.append({
            "cand_t": np.ascontiguousarray(cand_rows.T),
            "mm_t": mm_t,
        })

    nc = _get_program((w_pad, tuple(k_pads), tuple(cands), tuple(offs[:-1])))
    if _trace:
        _install_ntff_hook()
    res = run_bass_kernel_spmd(nc, in_maps, list(range(N_CORES)), trace=_trace)
    results = res.results
    if _trace:
        kernel.last_exec_time_ns = res.exec_time_ns
        kernel.last_results = results
        kernel.last_meta = (core_cls, [o for o in offs], idx_by_class)

    # ---------------- CPU bookkeeping (f64-exact decisions) ----------------
    norm_x = _f64_row_norms(instances)                       # [N]
    mem64 = memory.astype(np.float64)
    mm64 = mem64.mean(axis=1)                                # [C, D]
    norm_mm = np.einsum("cd,cd->c", mm64, mm64)              # [C]
    norm_slots = np.einsum("cbd,cbd->cb", mem64, mem64)      # [C, BANK]

    # --- argmin over classes ---
    # d2part[n, c] = |mm_c|^2 - 2 * dot(x_n, mm_c)   (|x|^2 is constant per row)
    d2part = np.empty((n_ins, N_CLASSES), dtype=np.float64)
    for core in range(N_CORES):
        dist = results[core]["dist"]  # [C, w_pad] f32
        for j, c in core_cls[core]:
            idx = idx_by_class[c]
            if len(idx) == 0:
                continue
            o = offs[j] + BANK
            d2part[idx, :] = norm_mm[None, :] - 2.0 * dist[:, o:o + len(idx)].T
    cls_prob = np.argmin(d2part, axis=1).astype(np.int32)
    part = np.partition(d2part, 1, axis=1)
    need = np.nonzero(part[:, 1] - part[:, 0] < TAU_DIST)[0]
    if len(need):
        # level 2: f32 sgemm
        d2f32 = (norm_mm[None, :].astype(np.float32)
                 - 2.0 * (instances[need] @ mem_mean32.T))
        cls_prob[need] = np.argmin(d2f32, axis=1).astype(np.int32)
        p2 = np.partition(d2f32.astype(np.float64), 1, axis=1)
        need2 = np.nonzero(p2[:, 1] - p2[:, 0] < TAU_DIST2)[0]
        if len(need2):
            xr = instances[need[need2]].astype(np.float64)
            d2ex = norm_mm[None, :] - 2.0 * (xr @ mm64.T)
            cls_prob[need[need2]] = np.argmin(d2ex, axis=1).astype(np.int32)
        kernel.n_refine_dist2 = len(need2)
    else:
        kernel.n_refine_dist2 = 0
    kernel.n_refine_dist = len(need)

    acc = np.float32(np.mean((cls_prob.astype(np.int64) == labels).astype(np.float32)))

    # --- sequential bank update, vectorized across classes in lockstep ---
    k_max = int(counts.max()) if n_ins else 0
    cand_max = max(cands)
    kp_max = max(k_pads)
    g_stack = np.zeros((N_CLASSES, cand_max, kp_max), dtype=np.float32)
    gcols = {}
    gw = 0
    for j in range(n_slots):
        for r0 in range(0, cands[j], P):
            gcols[(j, r0)] = gw
            gw += k_pads[j] - max(0, r0 - BANK)
    for core in range(N_CORES):
        gram = results[core]["gram"]  # [P, gw]
        for j, c in core_cls[core]:
            kp = k_pads[j]
            for r0 in range(0, cands[j], P):
                rows = min(P, cands[j] - r0)
                gc = gcols[(j, r0)]
                c_lo = max(0, r0 - BANK)
                g_stack[c, r0:r0 + rows, c_lo:kp] = gram[:rows,
                                                         gc:gc + kp - c_lo]

    src = np.tile(np.arange(BANK, dtype=np.int64), (N_CLASSES, 1))  # cand row idx
    snorm = norm_slots.copy()                                       # [C, BANK]
    pos = memory_pos.astype(np.int64).copy()
    n_refine_chain = 0
    for t in range(k_max):
        active = counts > t
        app = active & (pos < BANK)
        full = active & ~app
        if app.any():
            ac = np.nonzero(app)[0]
            j_new = pos[ac]
            g_idx = np.array([idx_by_class[c][t] for c in ac])
            src[ac, j_new] = BANK + t
            snorm[ac, j_new] = norm_x[g_idx]
            pos[ac] += 1
        if full.any():
            fc = np.nonzero(full)[0]
            dots = g_stack[fc[:, None], src[fc], t].astype(np.float64)  # [F, BANK]
            d2a = snorm[fc] - 2.0 * dots
            prt = np.partition(d2a, BANK - 2, axis=1)
            margin = prt[:, -1] - prt[:, -2]
            j_new = np.argmax(d2a, axis=1)
            for fi in np.nonzero(margin < TAU_GRAM)[0]:
                c = fc[fi]
                n_refine_chain += 1
                vecs = np.empty((BANK, D), dtype=np.float64)
                for jj in range(BANK):
                    s = src[c, jj]
                    if s < BANK:
                        vecs[jj] = mem64[c, s]
                    else:
                        vecs[jj] = instances[idx_by_class[c][s - BANK]]
                x_t = instances[idx_by_class[c][t]].astype(np.float64)
                d2ex = ((vecs - x_t[None, :]) ** 2).sum(axis=1)
                j_new[fi] = int(np.argmax(d2ex))
            g_idx = np.array([idx_by_class[c][t] for c in fc])
            src[fc, j_new] = BANK + t
            snorm[fc, j_new] = norm_x[g_idx]
    kernel.n_refine_chain = n_refine_chain

    new_mem = np.empty_like(memory)
    for c in range(N_CLASSES):
        for jj in range(BANK):
            s = src[c, jj]
            if s < BANK:
                new_mem[c, jj] = memory[c, s]
            else:
                new_mem[c, jj] = instances[idx_by_class[c][s - BANK]]
    new_pos = np.minimum(memory_pos.astype(np.int64) + counts, BANK).astype(
        memory_pos.dtype)

    return cls_prob, acc, new_mem, new_pos


# revision 47
# speedup vs baseline: 1.1866x; 1.0015x over previous
"""Trainium2 kernel for the scatter_memory problem.

Strategy (8 NeuronCores, expert-parallel over classes):
  * Classes are snake-assigned to 8 cores by descending instance count so the
    per-slot padded sizes are nearly uniform across cores (one SPMD program).
  * Each core receives a packed, transposed "candidates" matrix
    cand_T [D, W] (bf16): for each of its class slots, 10 memory-bank columns
    followed by that class's instances (original scan order), zero padded.
  * On device (per core):
      - dist[81, W]  = mem_mean @ cand: dot of every class mean with every
        candidate column -> drives the argmin over classes.
      - per class slot: gram[CAND, K] = cand_cls @ X_cls^T: dot tables that
        drive the order-dependent bank update.
    All matmuls bf16 inputs with fp32 PSUM accumulation.
  * On CPU: the tiny order-dependent bookkeeping. All comparisons use the
    device dot products; any decision whose margin is within TAU of the
    boundary is recomputed at higher precision (f32 sgemm, then f64), making
    every argmin/argmax decision exactly the true (f64) decision while the
    device does ~all the FLOPs and data movement. Decision margins for this
    problem's data are ~0.1-100 in squared-distance units; bf16 device dots
    are accurate to ~0.4, f32 to ~2e-4, so the two-level guard bands leave
    >5x safety at each level.

new_mem rows are bit-copies of input rows (instances / memory), so outputs
match the reference bit-exactly once every decision matches.
"""

import sys
import types
import numpy as np
import ml_dtypes

import concourse.mybir as mybir
import concourse.tile as tile
from concourse import bacc
from concourse.bass_utils import run_bass_kernel_spmd

N_CLASSES = 81
BANK = 10
D = 2048
N_CORES = 8
P = 128

TAU_DIST = 1.5    # bf16 distance d2 margin guard (measured dev err <= ~0.4)
TAU_DIST2 = 0.02  # f32 sgemm second-level guard (err ~2e-4)
TAU_GRAM = 6.0    # bf16 gram d2 margin guard

_PROGRAM_CACHE = {}


def _install_ntff_hook():
    """The agent image's antenv lacks axon_hooks; synthesize it so
    run_bass_kernel_spmd(trace=True) can profile via the axon .so."""
    name = "antenv.axon_hooks"
    mod = sys.modules.get(name)
    if mod is None:
        mod = types.ModuleType(name)
        mod._hook = None
        mod.set_axon_ntff_profile_hook = lambda h: setattr(mod, "_hook", h)
        mod.get_axon_ntff_profile_hook = lambda: mod._hook
        sys.modules[name] = mod
        try:
            import antenv
            antenv.axon_hooks = mod
        except ImportError:
            pass
    if mod.get_axon_ntff_profile_hook() is None:
        try:
            from trn_agent_boot.trn_boot import _ntff_profile_via_ctypes
            mod.set_axon_ntff_profile_hook(
                _ntff_profile_via_ctypes("/opt/axon/libaxon_pjrt.so"))
        except Exception:
            pass


def _assign_classes(counts):
    """Snake-assign classes (desc. by count) to cores; per-slot padded sizes."""
    order = np.argsort(-counts, kind="stable")
    n_slots = (N_CLASSES + N_CORES - 1) // N_CORES
    slot_classes = -np.ones((n_slots, N_CORES), dtype=np.int64)
    for r, cls in enumerate(order):
        j, i = divmod(r, N_CORES)
        core = i if (j % 2 == 0) else N_CORES - 1 - i
        slot_classes[j, core] = cls
    k_pad = []
    for j in range(n_slots):
        mx = max(int(counts[c]) if c >= 0 else 0 for c in slot_classes[j])
        k_pad.append(max(8, (mx + 7) // 8 * 8))
    return slot_classes, k_pad


def _build_program(w_pad, k_pads, cands, offs):
    n_slots = len(k_pads)
    kch = D // P
    nc = bacc.Bacc("TRN2", target_bir_lowering=False, debug=False)
    cand_dram = nc.dram_tensor("cand_t", [D, w_pad], mybir.dt.bfloat16,
                               kind="ExternalInput")
    # mem_mean, pre-arranged host-side to [P, KCH*C] so the DMA is contiguous
    mm_dram = nc.dram_tensor("mm_t", [P, kch * N_CLASSES], mybir.dt.bfloat16,
                             kind="ExternalInput")
    dist_dram = nc.dram_tensor("dist", [N_CLASSES, w_pad], mybir.dt.float32,
                               kind="ExternalOutput")
    # gram tables packed tight into one [P, sum(k_pads per rblock)] tensor:
    # rblock (j, r0) owns columns [gcol[j,r0], gcol[j,r0]+k_pads[j])
    rblocks = [(j, r0) for j in range(n_slots) for r0 in range(0, cands[j], P)]
    gcols = {}
    gw = 0
    for j, r0 in rblocks:
        gcols[(j, r0)] = gw
        gw += k_pads[j] - max(0, r0 - BANK)
    gram_dram = nc.dram_tensor("gram", [P, gw], mybir.dt.float32,
                               kind="ExternalOutput")

    n_db = (w_pad + 511) // 512  # distance column blocks
    db_sizes = [min(512, w_pad - b * 512) for b in range(n_db)]

    kh = kch // 2                  # gram/dist contraction halves
    n_sub = 2                      # column-split per chunk DMA (2-wave arrival)
    sub = (w_pad // n_sub + P - 1) // P * P

    with tile.TileContext(nc) as tc:
        with (
            tc.tile_pool(name="bfpool", bufs=kch) as bfpool,
            tc.tile_pool(name="mmpool", bufs=1) as mmpool,
            tc.tile_pool(name="opool", bufs=4) as opool,
            tc.tile_pool(name="daccpool", bufs=n_db) as daccpool,
            tc.tile_pool(name="goutpool", bufs=1) as goutpool,
            tc.tile_pool(name="gpartpool", bufs=len(rblocks)) as gpartpool,
            tc.tile_pool(name="dps", bufs=n_db, space="PSUM") as dps,
            tc.tile_pool(name="gps", bufs=3, space="PSUM") as gps,
        ):
            mm_sb = mmpool.tile([P, kch, N_CLASSES], mybir.dt.bfloat16)
            nc.sync.dma_start(mm_sb[:], mm_dram.rearrange("p (k c) -> p k c",
                                                          k=kch))

            # PE warmup: dummy matmuls on the (small, early) mm tile so the
            # HAM clock gate opens before the real work arrives and stays
            # open through the chunk-arrival gaps.
            warm_ps = gps.tile([P, 512], mybir.dt.float32, tag="gps",
                               name="warm_ps")
            mm_flat = mm_sb.rearrange("p k c -> p (k c)")

            def warm(n):
                for _ in range(n):
                    nc.tensor.matmul(warm_ps[:, :512], mm_flat[:, :128],
                                     mm_flat[:, :512], start=True, stop=True)

            warm(10)

            cand_bf = [bfpool.tile([P, w_pad], mybir.dt.bfloat16, tag="candbf",
                                   name=f"cand{k}") for k in range(kch)]

            def chunk_dma(k):
                for s in range(n_sub):
                    c0 = s * sub
                    cw = min(sub, w_pad - c0)
                    if cw > 0:
                        nc.sync.dma_start(
                            cand_bf[k][:, c0:c0 + cw],
                            cand_dram[k * P:(k + 1) * P, c0:c0 + cw])

            def gram_mms(j, r0, klo, khi, suffix, pool=None, tag="gps"):
                # Triangular skip: candidate row r0+i (instance r0+i-BANK) is
                # only read at steps t > r0+i-BANK, so rowblocks with r0 > 0
                # only need columns >= r0-BANK.
                o = offs[j]
                kp = k_pads[j]
                rows = min(P, cands[j] - r0)
                c_lo = max(0, r0 - BANK)
                gp = (pool or gps).tile([P, 512], mybir.dt.float32, tag=tag,
                                        name=f"gps{suffix}{j}_{r0}")[:rows,
                                                                     :kp - c_lo]
                for k in range(klo, khi):
                    nc.tensor.matmul(
                        gp,
                        cand_bf[k][:, o + r0:o + r0 + rows],
                        cand_bf[k][:, o + BANK + c_lo:o + BANK + kp],
                        start=(k == klo), stop=(k == khi - 1),
                    )
                return gp, rows, kp - c_lo

            # ---- chunks 0..kh-1 stream in; dist half-0 paced by arrival ----
            dist_ps0 = [dps.tile([P, 512], mybir.dt.float32, tag="dps",
                                 name=f"dps0_{b}")[:N_CLASSES, :db_sizes[b]]
                        for b in range(n_db)]
            for k in range(kh):
                chunk_dma(k)
                for b in range(n_db):
                    nc.tensor.matmul(
                        dist_ps0[b], mm_sb[:, k, :],
                        cand_bf[k][:, b * 512:b * 512 + db_sizes[b]],
                        start=(k == 0), stop=(k == kh - 1))
            # issue the second half's DMAs right away so queues stay busy
            for k in range(kh, kch):
                chunk_dma(k)
            # drain dist half-0 psum to SBUF accumulators
            dacc = []
            for b in range(n_db):
                da = daccpool.tile([N_CLASSES, 512], mybir.dt.float32,
                                   tag="dacc", name=f"dacc{b}")
                nc.scalar.copy(da[:, :db_sizes[b]], dist_ps0[b])
                dacc.append(da)

            # ---- gram half-A (chunks 0..kh-1) overlaps chunk kh..15 DMAs ----
            partials = {}
            ai = 0
            for j, r0 in rblocks:
                ai += 1
                pool, tag = (gps, "gps") if ai % 2 else (dps, "dps")
                gp, rows, kp = gram_mms(j, r0, 0, kh, "A", pool, tag)
                ga = gpartpool.tile([P, 512], mybir.dt.float32, tag="gpart",
                                    name=f"gpart{j}_{r0}")
                # alternate drain engine so neither ACT nor DVE paces PSUM reuse
                if ai % 2:
                    nc.scalar.copy(ga[:rows, :kp], gp)
                else:
                    nc.vector.tensor_copy(ga[:rows, :kp], gp)
                partials[(j, r0)] = ga

            # ---- dist half-1 (all chunks present by now) ----
            dist_ps1 = [dps.tile([P, 512], mybir.dt.float32, tag="dps",
                                 name=f"dps1_{b}")[:N_CLASSES, :db_sizes[b]]
                        for b in range(n_db)]
            for k in range(kh, kch):
                for b in range(n_db):
                    nc.tensor.matmul(
                        dist_ps1[b], mm_sb[:, k, :],
                        cand_bf[k][:, b * 512:b * 512 + db_sizes[b]],
                        start=(k == kh), stop=(k == kch - 1))
            for b in range(n_db):
                dsb = opool.tile([N_CLASSES, 512], mybir.dt.float32, tag="dout")
                nc.vector.tensor_add(dsb[:, :db_sizes[b]], dist_ps1[b],
                                     dacc[b][:, :db_sizes[b]])
                nc.sync.dma_start(dist_dram[:, b * 512:b * 512 + db_sizes[b]],
                                  dsb[:, :db_sizes[b]])

            # ---- gram half-B + combine into the packed output tile ----
            gout = goutpool.tile([P, gw], mybir.dt.float32)
            nc.any.memzero(gout[:])
            n_og = 11
            per_g = (len(rblocks) + n_og - 1) // n_og
            bi = 0
            for gi in range(n_og):
                grp = rblocks[gi * per_g:(gi + 1) * per_g]
                if not grp:
                    continue
                for j, r0 in grp:
                    # alternate PSUM pools: dist's dps banks are free by now,
                    # giving 8 in-flight accumulators instead of 3
                    bi += 1
                    pool, tag = (gps, "gps") if bi % 2 else (dps, "dps")
                    gp, rows, kp = gram_mms(j, r0, kh, kch, "B", pool, tag)
                    ga = partials[(j, r0)]
                    gc = gcols[(j, r0)]
                    nc.vector.tensor_add(gout[:rows, gc:gc + kp], gp,
                                         ga[:rows, :kp])
                c_lo = gcols[grp[0]]
                j, r0 = grp[-1]
                c_hi = gcols[(j, r0)] + k_pads[j] - max(0, r0 - BANK)
                nc.sync.dma_start(gram_dram[:, c_lo:c_hi], gout[:, c_lo:c_hi])
    nc.compile()
    return nc


def _get_program(key_args):
    key = repr(key_args)
    if key not in _PROGRAM_CACHE:
        _PROGRAM_CACHE[key] = _build_program(*key_args)
    return _PROGRAM_CACHE[key]


def _f64_row_norms(x, chunk=4096):
    out = np.empty(x.shape[0], dtype=np.float64)
    for i in range(0, x.shape[0], chunk):
        xi = x[i:i + chunk].astype(np.float64)
        out[i:i + chunk] = np.einsum("ij,ij->i", xi, xi)
    return out


def kernel(instances, instance_labels, memory, memory_pos, _trace=False):
    instances = np.ascontiguousarray(instances, dtype=np.float32)
    labels = np.asarray(instance_labels).astype(np.int64)
    memory = np.ascontiguousarray(memory, dtype=np.float32)
    memory_pos = np.asarray(memory_pos)
    n_ins = instances.shape[0]

    counts = np.bincount(labels, minlength=N_CLASSES)
    slot_classes, k_pads = _assign_classes(counts)
    n_slots = len(k_pads)
    cands = [BANK + kp for kp in k_pads]
    offs = np.concatenate([[0], np.cumsum(cands)]).astype(int)
    w = int(offs[-1])
    w_pad = (w + P - 1) // P * P

    # Per-class instance index lists (original scan order).
    sort_idx = np.argsort(labels, kind="stable")
    cls_starts = np.concatenate([[0], np.cumsum(counts)]).astype(int)
    idx_by_class = [sort_idx[cls_starts[c]:cls_starts[c + 1]] for c in range(N_CLASSES)]

    ins_bf = instances.astype(ml_dtypes.bfloat16)
    mem_bf = memory.astype(ml_dtypes.bfloat16)
    mem_mean32 = memory.mean(axis=1)  # [C, D] f32
    # [D, C] -> chunked [P, KCH*C] so the device DMA is one contiguous copy
    mm_t = np.ascontiguousarray(
        mem_mean32.T.astype(ml_dtypes.bfloat16)
        .reshape(D // P, P, N_CLASSES).transpose(1, 0, 2)
        .reshape(P, (D // P) * N_CLASSES))

    # Build per-core packed transposed candidates (bf16).
    in_maps = []
    core_cls = [[] for _ in range(N_CORES)]  # (slot j, class c)
    for core in range(N_CORES):
        cand_rows = np.zeros((w_pad, D), dtype=ml_dtypes.bfloat16)
        for j in range(n_slots):
            c = slot_classes[j][core]
            if c < 0:
                continue
            core_cls[core].append((j, int(c)))
            o = offs[j]
            cand_rows[o:o + BANK] = mem_bf[c]
            kc = int(counts[c])
            cand_rows[o + BANK:o + BANK + kc] = ins_bf[idx_by_class[c]]
        in_maps.append({
            "cand_t": np.ascontiguousarray(cand_rows.T),
            "mm_t": mm_t,
        })

    nc = _get_program((w_pad, tuple(k_pads), tuple(cands), tuple(offs[:-1])))
    if _trace:
        _install_ntff_hook()
    res = run_bass_kernel_spmd(nc, in_maps, list(range(N_CORES)), trace=_trace)
    results = res.results
    if _trace:
        kernel.last_exec_time_ns = res.exec_time_ns
        kernel.last_results = results
        kernel.last_meta = (core_cls, [o for o in offs], idx_by_class)

    # ---------------- CPU bookkeeping (f64-exact decisions) ----------------
    norm_x = _f64_row_norms(instances)                       # [N]
    mem64 = memory.astype(np.float64)
    mm64 = mem64.mean(axis=1)                                # [C, D]
    norm_mm = np.einsum("cd,cd->c", mm64, mm64)              # [C]
    norm_slots = np.einsum("cbd,cbd->cb", mem64, mem64)      # [C, BANK]

    # --- argmin over classes ---
    # d2part[n, c] = |mm_c|^2 - 2 * dot(x_n, mm_c)   (|x|^2 is constant per row)
    d2part = np.empty((n_ins, N_CLASSES), dtype=np.float64)
    for core in range(N_CORES):
        dist = results[core]["dist"]  # [C, w_pad] f32
        for j, c in core_cls[core]:
            idx = idx_by_class[c]
            if len(idx) == 0:
                continue
            o = offs[j] + BANK
            d2part[idx, :] = norm_mm[None, :] - 2.0 * dist[:, o:o + len(idx)].T
    cls_prob = np.argmin(d2part, axis=1).astype(np.int32)
    part = np.partition(d2part, 1, axis=1)
    need = np.nonzero(part[:, 1] - part[:, 0] < TAU_DIST)[0]
    if len(need):
        # level 2: f32 sgemm
        d2f32 = (norm_mm[None, :].astype(np.float32)
                 - 2.0 * (instances[need] @ mem_mean32.T))
        cls_prob[need] = np.argmin(d2f32, axis=1).astype(np.int32)
        p2 = np.partition(d2f32.astype(np.float64), 1, axis=1)
        need2 = np.nonzero(p2[:, 1] - p2[:, 0] < TAU_DIST2)[0]
        if len(need2):
            xr = instances[need[need2]].astype(np.float64)
            d2ex = norm_mm[None, :] - 2.0 * (xr @ mm64.T)
            cls_prob[need[need2]] = np.argmin(d2ex, axis=1).astype(np.int32)
        kernel.n_refine_dist2 = len(need2)
    else:
        kernel.n_refine_dist2 = 0
    kernel.n_refine_dist = len(need)

    acc = np.float32(np.mean((cls_prob.astype(np.int64) == labels).astype(np.float32)))

    # --- sequential bank update, vectorized across classes in lockstep ---
    k_max = int(counts.max()) if n_ins else 0
    cand_max = max(cands)
    kp_max = max(k_pads)
    g_stack = np.zeros((N_CLASSES, cand_max, kp_max), dtype=np.float32)
    gcols = {}
    gw = 0
    for j in range(n_slots):
        for r0 in range(0, cands[j], P):
            gcols[(j, r0)] = gw
            gw += k_pads[j] - max(0, r0 - BANK)
    for core in range(N_CORES):
        gram = results[core]["gram"]  # [P, gw]
        for j, c in core_cls[core]:
            kp = k_pads[j]
            for r0 in range(0, cands[j], P):
                rows = min(P, cands[j] - r0)
                gc = gcols[(j, r0)]
                c_lo = max(0, r0 - BANK)
                g_stack[c, r0:r0 + rows, c_lo:kp] = gram[:rows,
                                                         gc:gc + kp - c_lo]

    src = np.tile(np.arange(BANK, dtype=np.int64), (N_CLASSES, 1))  # cand row idx
    snorm = norm_slots.copy()                                       # [C, BANK]
    pos = memory_pos.astype(np.int64).copy()
    n_refine_chain = 0
    for t in range(k_max):
        active = counts > t
        app = active & (pos < BANK)
        full = active & ~app
        if app.any():
            ac = np.nonzero(app)[0]
            j_new = pos[ac]
            g_idx = np.array([idx_by_class[c][t] for c in ac])
            src[ac, j_new] = BANK + t
            snorm[ac, j_new] = norm_x[g_idx]
            pos[ac] += 1
        if full.any():
            fc = np.nonzero(full)[0]
            dots = g_stack[fc[:, None], src[fc], t].astype(np.float64)  # [F, BANK]
            d2a = snorm[fc] - 2.0 * dots
            prt = np.partition(d2a, BANK - 2, axis=1)
            margin = prt[:, -1] - prt[:, -2]
            j_new = np.argmax(d2a, axis=1)
            for fi in np.nonzero(margin < TAU_GRAM)[0]:
                c = fc[fi]
                n_refine_chain += 1
                vecs = np.empty((BANK, D), dtype=np.float64)
                for jj in range(BANK):
                    s = src[c, jj]
                    if s < BANK:
                        vecs[jj] = mem64[c, s]
                    else:
                        vecs[jj] = instances[idx_by_class[c][s - BANK]]
                x_t = instances[idx_by_class[c][t]].astype(np.float64)
                d2ex = ((vecs - x_t[None, :]) ** 2).sum(axis=1)
                j_new[fi] = int(np.argmax(d2ex))
            g_idx = np.array([idx_by_class[c][t] for c in fc])
            src[fc, j_new] = BANK + t
            snorm[fc, j_new] = norm_x[g_idx]
    kernel.n_refine_chain = n_refine_chain

    new_mem = np.empty_like(memory)
    for c in range(N_CLASSES):
        for jj in range(BANK):
            s = src[c, jj]
            if s < BANK:
                new_mem[c, jj] = memory[c, s]
            else:
                new_mem[c, jj] = instances[idx_by_class[c][s - BANK]]
    new_pos = np.minimum(memory_pos.astype(np.int64) + counts, BANK).astype(
        memory_pos.dtype)

    return cls_prob, acc, new_mem, new_pos


# revision 48
# speedup vs baseline: 1.1965x; 1.0083x over previous
"""Trainium2 kernel for the scatter_memory problem.

Strategy (8 NeuronCores, expert-parallel over classes):
  * Classes are snake-assigned to 8 cores by descending instance count so the
    per-slot padded sizes are nearly uniform across cores (one SPMD program).
  * Each core receives a packed, transposed "candidates" matrix
    cand_T [D, W] (bf16): for each of its class slots, 10 memory-bank columns
    followed by that class's instances (original scan order), zero padded.
  * On device (per core):
      - dist[81, W]  = mem_mean @ cand: dot of every class mean with every
        candidate column -> drives the argmin over classes.
      - per class slot: gram[CAND, K] = cand_cls @ X_cls^T: dot tables that
        drive the order-dependent bank update.
    All matmuls bf16 inputs with fp32 PSUM accumulation.
  * On CPU: the tiny order-dependent bookkeeping. All comparisons use the
    device dot products; any decision whose margin is within TAU of the
    boundary is recomputed at higher precision (f32 sgemm, then f64), making
    every argmin/argmax decision exactly the true (f64) decision while the
    device does ~all the FLOPs and data movement. Decision margins for this
    problem's data are ~0.1-100 in squared-distance units; bf16 device dots
    are accurate to ~0.4, f32 to ~2e-4, so the two-level guard bands leave
    >5x safety at each level.

new_mem rows are bit-copies of input rows (instances / memory), so outputs
match the reference bit-exactly once every decision matches.
"""

import sys
import types
import numpy as np
import ml_dtypes

import concourse.mybir as mybir
import concourse.tile as tile
from concourse import bacc
from concourse.bass_utils import run_bass_kernel_spmd

N_CLASSES = 81
BANK = 10
D = 2048
N_CORES = 8
P = 128

TAU_DIST = 1.5    # bf16 distance d2 margin guard (measured dev err <= ~0.4)
TAU_DIST2 = 0.02  # f32 sgemm second-level guard (err ~2e-4)
TAU_GRAM = 6.0    # bf16 gram d2 margin guard

_PROGRAM_CACHE = {}


def _install_ntff_hook():
    """The agent image's antenv lacks axon_hooks; synthesize it so
    run_bass_kernel_spmd(trace=True) can profile via the axon .so."""
    name = "antenv.axon_hooks"
    mod = sys.modules.get(name)
    if mod is None:
        mod = types.ModuleType(name)
        mod._hook = None
        mod.set_axon_ntff_profile_hook = lambda h: setattr(mod, "_hook", h)
        mod.get_axon_ntff_profile_hook = lambda: mod._hook
        sys.modules[name] = mod
        try:
            import antenv
            antenv.axon_hooks = mod
        except ImportError:
            pass
    if mod.get_axon_ntff_profile_hook() is None:
        try:
            from trn_agent_boot.trn_boot import _ntff_profile_via_ctypes
            mod.set_axon_ntff_profile_hook(
                _ntff_profile_via_ctypes("/opt/axon/libaxon_pjrt.so"))
        except Exception:
            pass


def _assign_classes(counts):
    """Snake-assign classes (desc. by count) to cores; per-slot padded sizes."""
    order = np.argsort(-counts, kind="stable")
    n_slots = (N_CLASSES + N_CORES - 1) // N_CORES
    slot_classes = -np.ones((n_slots, N_CORES), dtype=np.int64)
    for r, cls in enumerate(order):
        j, i = divmod(r, N_CORES)
        core = i if (j % 2 == 0) else N_CORES - 1 - i
        slot_classes[j, core] = cls
    k_pad = []
    for j in range(n_slots):
        mx = max(int(counts[c]) if c >= 0 else 0 for c in slot_classes[j])
        k_pad.append(max(8, (mx + 7) // 8 * 8))
    return slot_classes, k_pad


def _build_program(w_pad, k_pads, cands, offs):
    n_slots = len(k_pads)
    kch = D // P
    nc = bacc.Bacc("TRN2", target_bir_lowering=False, debug=False)
    cand_dram = nc.dram_tensor("cand_t", [D, w_pad], mybir.dt.bfloat16,
                               kind="ExternalInput")
    # mem_mean, pre-arranged host-side to [P, KCH*C] so the DMA is contiguous
    mm_dram = nc.dram_tensor("mm_t", [P, kch * N_CLASSES], mybir.dt.bfloat16,
                             kind="ExternalInput")
    dist_dram = nc.dram_tensor("dist", [N_CLASSES, w_pad], mybir.dt.float32,
                               kind="ExternalOutput")
    # gram tables packed tight into one [P, sum(k_pads per rblock)] tensor:
    # rblock (j, r0) owns columns [gcol[j,r0], gcol[j,r0]+k_pads[j])
    rblocks = [(j, r0) for j in range(n_slots) for r0 in range(0, cands[j], P)]
    gcols = {}
    gw = 0
    for j, r0 in rblocks:
        gcols[(j, r0)] = gw
        gw += k_pads[j] - max(0, r0 - BANK)
    gram_dram = nc.dram_tensor("gram", [P, gw], mybir.dt.float32,
                               kind="ExternalOutput")

    n_db = (w_pad + 511) // 512  # distance column blocks
    db_sizes = [min(512, w_pad - b * 512) for b in range(n_db)]

    kh = kch // 2                  # gram/dist contraction halves
    n_sub = 2                      # column-split per chunk DMA (2-wave arrival)
    sub = (w_pad // n_sub + P - 1) // P * P

    with tile.TileContext(nc) as tc:
        with (
            tc.tile_pool(name="bfpool", bufs=kch) as bfpool,
            tc.tile_pool(name="mmpool", bufs=1) as mmpool,
            tc.tile_pool(name="opool", bufs=4) as opool,
            tc.tile_pool(name="daccpool", bufs=n_db) as daccpool,
            tc.tile_pool(name="goutpool", bufs=1) as goutpool,
            tc.tile_pool(name="gpartpool", bufs=len(rblocks)) as gpartpool,
            tc.tile_pool(name="dps", bufs=n_db, space="PSUM") as dps,
            tc.tile_pool(name="gps", bufs=3, space="PSUM") as gps,
        ):
            mm_sb = mmpool.tile([P, kch, N_CLASSES], mybir.dt.bfloat16)
            nc.sync.dma_start(mm_sb[:], mm_dram.rearrange("p (k c) -> p k c",
                                                          k=kch))

            # PE warmup: dummy matmuls on a zeroed scratch tile (no DMA
            # dependency, starts at t~0) so the HAM clock gate opens before
            # the real work arrives.
            warm_ps = gps.tile([P, 512], mybir.dt.float32, tag="gps",
                               name="warm_ps")
            warm_sb = mmpool.tile([P, 512], mybir.dt.bfloat16, name="warm_sb")
            nc.vector.memset(warm_sb[:], 0.0)

            def warm(n):
                for _ in range(n):
                    nc.tensor.matmul(warm_ps[:, :512], warm_sb[:, :128],
                                     warm_sb[:, :512], start=True, stop=True)

            warm(10)

            cand_bf = [bfpool.tile([P, w_pad], mybir.dt.bfloat16, tag="candbf",
                                   name=f"cand{k}") for k in range(kch)]

            def chunk_dma(k):
                for s in range(n_sub):
                    c0 = s * sub
                    cw = min(sub, w_pad - c0)
                    if cw > 0:
                        nc.sync.dma_start(
                            cand_bf[k][:, c0:c0 + cw],
                            cand_dram[k * P:(k + 1) * P, c0:c0 + cw])

            def gram_mms(j, r0, klo, khi, suffix, pool=None, tag="gps"):
                # Triangular skip: candidate row r0+i (instance r0+i-BANK) is
                # only read at steps t > r0+i-BANK, so rowblocks with r0 > 0
                # only need columns >= r0-BANK.
                o = offs[j]
                kp = k_pads[j]
                rows = min(P, cands[j] - r0)
                c_lo = max(0, r0 - BANK)
                gp = (pool or gps).tile([P, 512], mybir.dt.float32, tag=tag,
                                        name=f"gps{suffix}{j}_{r0}")[:rows,
                                                                     :kp - c_lo]
                for k in range(klo, khi):
                    nc.tensor.matmul(
                        gp,
                        cand_bf[k][:, o + r0:o + r0 + rows],
                        cand_bf[k][:, o + BANK + c_lo:o + BANK + kp],
                        start=(k == klo), stop=(k == khi - 1),
                    )
                return gp, rows, kp - c_lo

            # ---- chunks 0..kh-1 stream in; dist half-0 paced by arrival ----
            dist_ps0 = [dps.tile([P, 512], mybir.dt.float32, tag="dps",
                                 name=f"dps0_{b}")[:N_CLASSES, :db_sizes[b]]
                        for b in range(n_db)]
            for k in range(kh):
                chunk_dma(k)
                for b in range(n_db):
                    nc.tensor.matmul(
                        dist_ps0[b], mm_sb[:, k, :],
                        cand_bf[k][:, b * 512:b * 512 + db_sizes[b]],
                        start=(k == 0), stop=(k == kh - 1))
            # issue the second half's DMAs right away so queues stay busy
            for k in range(kh, kch):
                chunk_dma(k)
            # drain dist half-0 psum to SBUF accumulators
            dacc = []
            for b in range(n_db):
                da = daccpool.tile([N_CLASSES, 512], mybir.dt.float32,
                                   tag="dacc", name=f"dacc{b}")
                nc.scalar.copy(da[:, :db_sizes[b]], dist_ps0[b])
                dacc.append(da)

            # ---- gram half-A (chunks 0..kh-1) overlaps chunk kh..15 DMAs ----
            partials = {}
            ai = 0
            for j, r0 in rblocks:
                ai += 1
                pool, tag = (gps, "gps") if ai % 2 else (dps, "dps")
                gp, rows, kp = gram_mms(j, r0, 0, kh, "A", pool, tag)
                ga = gpartpool.tile([P, 512], mybir.dt.float32, tag="gpart",
                                    name=f"gpart{j}_{r0}")
                # alternate drain engine so neither ACT nor DVE paces PSUM reuse
                if ai % 2:
                    nc.scalar.copy(ga[:rows, :kp], gp)
                else:
                    nc.vector.tensor_copy(ga[:rows, :kp], gp)
                partials[(j, r0)] = ga

            # ---- dist half-1 (all chunks present by now) ----
            dist_ps1 = [dps.tile([P, 512], mybir.dt.float32, tag="dps",
                                 name=f"dps1_{b}")[:N_CLASSES, :db_sizes[b]]
                        for b in range(n_db)]
            for k in range(kh, kch):
                for b in range(n_db):
                    nc.tensor.matmul(
                        dist_ps1[b], mm_sb[:, k, :],
                        cand_bf[k][:, b * 512:b * 512 + db_sizes[b]],
                        start=(k == kh), stop=(k == kch - 1))
            for b in range(n_db):
                dsb = opool.tile([N_CLASSES, 512], mybir.dt.float32, tag="dout")
                nc.vector.tensor_add(dsb[:, :db_sizes[b]], dist_ps1[b],
                                     dacc[b][:, :db_sizes[b]])
                nc.sync.dma_start(dist_dram[:, b * 512:b * 512 + db_sizes[b]],
                                  dsb[:, :db_sizes[b]])

            # ---- gram half-B + combine into the packed output tile ----
            gout = goutpool.tile([P, gw], mybir.dt.float32)
            nc.any.memzero(gout[:])
            n_og = 11
            per_g = (len(rblocks) + n_og - 1) // n_og
            bi = 0
            for gi in range(n_og):
                grp = rblocks[gi * per_g:(gi + 1) * per_g]
                if not grp:
                    continue
                for j, r0 in grp:
                    # alternate PSUM pools: dist's dps banks are free by now,
                    # giving 8 in-flight accumulators instead of 3
                    bi += 1
                    pool, tag = (gps, "gps") if bi % 2 else (dps, "dps")
                    gp, rows, kp = gram_mms(j, r0, kh, kch, "B", pool, tag)
                    ga = partials[(j, r0)]
                    gc = gcols[(j, r0)]
                    nc.vector.tensor_add(gout[:rows, gc:gc + kp], gp,
                                         ga[:rows, :kp])
                c_lo = gcols[grp[0]]
                j, r0 = grp[-1]
                c_hi = gcols[(j, r0)] + k_pads[j] - max(0, r0 - BANK)
                nc.sync.dma_start(gram_dram[:, c_lo:c_hi], gout[:, c_lo:c_hi])
    nc.compile()
    return nc


def _get_program(key_args):
    key = repr(key_args)
    if key not in _PROGRAM_CACHE:
        _PROGRAM_CACHE[key] = _build_program(*key_args)
    return _PROGRAM_CACHE[key]


def _f64_row_norms(x, chunk=4096):
    out = np.empty(x.shape[0], dtype=np.float64)
    for i in range(0, x.shape[0], chunk):
        xi = x[i:i + chunk].astype(np.float64)
        out[i:i + chunk] = np.einsum("ij,ij->i", xi, xi)
    return out


def kernel(instances, instance_labels, memory, memory_pos, _trace=False):
    instances = np.ascontiguousarray(instances, dtype=np.float32)
    labels = np.asarray(instance_labels).astype(np.int64)
    memory = np.ascontiguousarray(memory, dtype=np.float32)
    memory_pos = np.asarray(memory_pos)
    n_ins = instances.shape[0]

    counts = np.bincount(labels, minlength=N_CLASSES)
    slot_classes, k_pads = _assign_classes(counts)
    n_slots = len(k_pads)
    cands = [BANK + kp for kp in k_pads]
    offs = np.concatenate([[0], np.cumsum(cands)]).astype(int)
    w = int(offs[-1])
    w_pad = (w + P - 1) // P * P

    # Per-class instance index lists (original scan order).
    sort_idx = np.argsort(labels, kind="stable")
    cls_starts = np.concatenate([[0], np.cumsum(counts)]).astype(int)
    idx_by_class = [sort_idx[cls_starts[c]:cls_starts[c + 1]] for c in range(N_CLASSES)]

    ins_bf = instances.astype(ml_dtypes.bfloat16)
    mem_bf = memory.astype(ml_dtypes.bfloat16)
    mem_mean32 = memory.mean(axis=1)  # [C, D] f32
    # [D, C] -> chunked [P, KCH*C] so the device DMA is one contiguous copy
    mm_t = np.ascontiguousarray(
        mem_mean32.T.astype(ml_dtypes.bfloat16)
        .reshape(D // P, P, N_CLASSES).transpose(1, 0, 2)
        .reshape(P, (D // P) * N_CLASSES))

    # Build per-core packed transposed candidates (bf16).
    in_maps = []
    core_cls = [[] for _ in range(N_CORES)]  # (slot j, class c)
    for core in range(N_CORES):
        cand_rows = np.zeros((w_pad, D), dtype=ml_dtypes.bfloat16)
        for j in range(n_slots):
            c = slot_classes[j][core]
            if c < 0:
                continue
            core_cls[core].append((j, int(c)))
            o = offs[j]
            cand_rows[o:o + BANK] = mem_bf[c]
            kc = int(counts[c])
            cand_rows[o + BANK:o + BANK + kc] = ins_bf[idx_by_class[c]]
        in_maps.append({
            "cand_t": np.ascontiguousarray(cand_rows.T),
            "mm_t": mm_t,
        })

    nc = _get_program((w_pad, tuple(k_pads), tuple(cands), tuple(offs[:-1])))
    if _trace:
        _install_ntff_hook()
    res = run_bass_kernel_spmd(nc, in_maps, list(range(N_CORES)), trace=_trace)
    results = res.results
    if _trace:
        kernel.last_exec_time_ns = res.exec_time_ns
        kernel.last_results = results
        kernel.last_meta = (core_cls, [o for o in offs], idx_by_class)

    # ---------------- CPU bookkeeping (f64-exact decisions) ----------------
    norm_x = _f64_row_norms(instances)                       # [N]
    mem64 = memory.astype(np.float64)
    mm64 = mem64.mean(axis=1)                                # [C, D]
    norm_mm = np.einsum("cd,cd->c", mm64, mm64)              # [C]
    norm_slots = np.einsum("cbd,cbd->cb", mem64, mem64)      # [C, BANK]

    # --- argmin over classes ---
    # d2part[n, c] = |mm_c|^2 - 2 * dot(x_n, mm_c)   (|x|^2 is constant per row)
    d2part = np.empty((n_ins, N_CLASSES), dtype=np.float64)
    for core in range(N_CORES):
        dist = results[core]["dist"]  # [C, w_pad] f32
        for j, c in core_cls[core]:
            idx = idx_by_class[c]
            if len(idx) == 0:
                continue
            o = offs[j] + BANK
            d2part[idx, :] = norm_mm[None, :] - 2.0 * dist[:, o:o + len(idx)].T
    cls_prob = np.argmin(d2part, axis=1).astype(np.int32)
    part = np.partition(d2part, 1, axis=1)
    need = np.nonzero(part[:, 1] - part[:, 0] < TAU_DIST)[0]
    if len(need):
        # level 2: f32 sgemm
        d2f32 = (norm_mm[None, :].astype(np.float32)
                 - 2.0 * (instances[need] @ mem_mean32.T))
        cls_prob[need] = np.argmin(d2f32, axis=1).astype(np.int32)
        p2 = np.partition(d2f32.astype(np.float64), 1, axis=1)
        need2 = np.nonzero(p2[:, 1] - p2[:, 0] < TAU_DIST2)[0]
        if len(need2):
            xr = instances[need[need2]].astype(np.float64)
            d2ex = norm_mm[None, :] - 2.0 * (xr @ mm64.T)
            cls_prob[need[need2]] = np.argmin(d2ex, axis=1).astype(np.int32)
        kernel.n_refine_dist2 = len(need2)
    else:
        kernel.n_refine_dist2 = 0
    kernel.n_refine_dist = len(need)

    acc = np.float32(np.mean((cls_prob.astype(np.int64) == labels).astype(np.float32)))

    # --- sequential bank update, vectorized across classes in lockstep ---
    k_max = int(counts.max()) if n_ins else 0
    cand_max = max(cands)
    kp_max = max(k_pads)
    g_stack = np.zeros((N_CLASSES, cand_max, kp_max), dtype=np.float32)
    gcols = {}
    gw = 0
    for j in range(n_slots):
        for r0 in range(0, cands[j], P):
            gcols[(j, r0)] = gw
            gw += k_pads[j] - max(0, r0 - BANK)
    for core in range(N_CORES):
        gram = results[core]["gram"]  # [P, gw]
        for j, c in core_cls[core]:
            kp = k_pads[j]
            for r0 in range(0, cands[j], P):
                rows = min(P, cands[j] - r0)
                gc = gcols[(j, r0)]
                c_lo = max(0, r0 - BANK)
                g_stack[c, r0:r0 + rows, c_lo:kp] = gram[:rows,
                                                         gc:gc + kp - c_lo]

    src = np.tile(np.arange(BANK, dtype=np.int64), (N_CLASSES, 1))  # cand row idx
    snorm = norm_slots.copy()                                       # [C, BANK]
    pos = memory_pos.astype(np.int64).copy()
    n_refine_chain = 0
    for t in range(k_max):
        active = counts > t
        app = active & (pos < BANK)
        full = active & ~app
        if app.any():
            ac = np.nonzero(app)[0]
            j_new = pos[ac]
            g_idx = np.array([idx_by_class[c][t] for c in ac])
            src[ac, j_new] = BANK + t
            snorm[ac, j_new] = norm_x[g_idx]
            pos[ac] += 1
        if full.any():
            fc = np.nonzero(full)[0]
            dots = g_stack[fc[:, None], src[fc], t].astype(np.float64)  # [F, BANK]
            d2a = snorm[fc] - 2.0 * dots
            prt = np.partition(d2a, BANK - 2, axis=1)
            margin = prt[:, -1] - prt[:, -2]
            j_new = np.argmax(d2a, axis=1)
            for fi in np.nonzero(margin < TAU_GRAM)[0]:
                c = fc[fi]
                n_refine_chain += 1
                vecs = np.empty((BANK, D), dtype=np.float64)
                for jj in range(BANK):
                    s = src[c, jj]
                    if s < BANK:
                        vecs[jj] = mem64[c, s]
                    else:
                        vecs[jj] = instances[idx_by_class[c][s - BANK]]
                x_t = instances[idx_by_class[c][t]].astype(np.float64)
                d2ex = ((vecs - x_t[None, :]) ** 2).sum(axis=1)
                j_new[fi] = int(np.argmax(d2ex))
            g_idx = np.array([idx_by_class[c][t] for c in fc])
            src[fc, j_new] = BANK + t
            snorm[fc, j_new] = norm_x[g_idx]
    kernel.n_refine_chain = n_refine_chain

    new_mem = np.empty_like(memory)
    for c in range(N_CLASSES):
        for jj in range(BANK):
            s = src[c, jj]
            if s < BANK:
                new_mem[c, jj] = memory[c, s]
            else:
                new_mem[c, jj] = instances[idx_by_class[c][s - BANK]]
    new_pos = np.minimum(memory_pos.astype(np.int64) + counts, BANK).astype(
        memory_pos.dtype)

    return cls_prob, acc, new_mem, new_pos
